# revision 20
# baseline (speedup 1.0000x reference)
"""Trainium2 Bass kernel for nn_BoxRegression_42125039239326 (YOLOv3 loss).

Contract: kernel(**inputs) takes FULL inputs (x:[32,15,256,256] f32,
targets:[4096,6] f32) and returns the full reference output tuple
(output:[32,196608,5] f32, loss_layer: f32 scalar, metrics:[12] f32).

Strategy (data-parallel over batch, 8 cores x 4 batches):
  Device (per core): the memory-bound full-grid work --
    - elementwise transform of x into `output` (sigmoid/exp + grid offsets,
      interleaved to channel-last), using one ACT table set (exp_and_others):
      sigmoid(c) = 0.5 + 0.5*tanh(c/2); exp(w)*anchor = exp(w + ln(anchor)).
    - full-grid reduction: count(conf>0) (== count(pred_conf>0.5), the
      precision metric's denominator).
  Host: O(T)=4096-sparse per-target work (build_targets scatter logic and
    the masked means, which are all sparse:

    NOTE on this XLA backend (neuron), `.at[...].min(keep)` scatter-min
    ZEROES all un-touched cells (verified empirically), so the reference's
    no_obj_mask is nonzero only at target-touched cells (~8k) -- the
    "no-obj" losses are therefore sparse sums over those cells, not
    full-grid reductions.  The unary transcendentals for the sparse values
    go through jnp so they match the reference's XLA lowerings bitwise.)
"""

import math

import numpy as np

# ---------------------------------------------------------------------------
# constants (hardcoded per problem spec)
B, A, G, T = 32, 3, 256, 4096
N_CORES = 8
B_LOC = B // N_CORES
P, Q = 128, 512          # SBUF tile: 128 partitions x 512 cells (2 grid rows)
NCH = 5
ANCHORS = np.array([[116.0, 90.0], [156.0, 198.0], [373.0, 326.0]], np.float32)
IMAGE_SIZE = 2048.0
STRIDE = IMAGE_SIZE / G          # 8.0
IGNORE_THRES = 0.5
OBJ_SCALE = 100.0
NO_OBJ_SCALE = 1.0
EPS = 1e-16
BCE_EPS = 1e-12
N_ALL = B * A * G * G


def _split_multiwaits(nc):
    """Walrus in this container allows 1 sem-wait per instruction; Tile's
    tail drain can carry several.  Splice extra waits onto preceding NoOps."""
    import concourse.mybir as mybir
    import bass_rust

    for f in nc.m.functions:
        for bb in f.blocks:
            new_insts = []
            for ins in bb.instructions:
                si = ins.sync_info
                if si is not None and si.on_wait is not None and len(si.on_wait) > 1:
                    waits = list(si.on_wait)
                    for i, w in enumerate(waits[:-1]):
                        nop = mybir.InstNoOp(name=f"{ins.name}-wsplit{i}")
                        nop.engine = ins.engine
                        nop.sync_info = bass_rust.SyncInfo(on_wait=[w], on_update=[])
                        new_insts.append(nop)
                    ins.sync_info = bass_rust.SyncInfo(
                        on_wait=[waits[-1]], on_update=list(si.on_update or [])
                    )
                new_insts.append(ins)
            bb.instructions = new_insts


_PROG = None


def _build_program():
    import concourse.bass as bass
    import concourse.mybir as mybir
    from concourse.tile import TileContext

    AF = mybir.ActivationFunctionType
    OP = mybir.AluOpType
    f32 = mybir.dt.float32

    nc = bass.Bass()
    x = nc.dram_tensor("x", [B_LOC, A * NCH, G, G], f32, kind="ExternalInput")
    grids = nc.dram_tensor("grids", [P, 2 * Q + 2 * A], f32, kind="ExternalInput")
    out = nc.dram_tensor("out", [B_LOC, A * G * G, NCH], f32, kind="ExternalOutput")
    accd = nc.dram_tensor("acc", [B_LOC, P, A], f32, kind="ExternalOutput")

    xf = x.rearrange("b c h w -> b c (h w)")

    with TileContext(nc) as tc:
        with tc.tile_pool(name="const", bufs=1) as cpool, \
             tc.tile_pool(name="inp", bufs=4) as ipool, \
             tc.tile_pool(name="outp", bufs=4) as opool, \
             tc.tile_pool(name="scr", bufs=3) as spool:
            gt = cpool.tile([P, 2 * Q + 2 * A], f32)
            nc.gpsimd.dma_start(gt[:], grids[:])
            gx = gt[:, 0:Q]          # 8*gx + 4
            gy = gt[:, Q:2 * Q]      # 8*gy + 4

            for b in range(B_LOC):
                acc_t = spool.tile([P, A], f32, tag="acc")
                for a in range(A):
                    # one 1.25 MB input DMA per (b, a) on the SP HWDGE ring;
                    # outputs ride SWDGE (gpsimd) so reads and writes stream
                    # concurrently on the 16 SDMA engines.
                    in_t = ipool.tile([P, NCH * Q], f32, tag="in")
                    nc.gpsimd.dma_start(
                        in_t[:].rearrange("p (c q) -> p c q", q=Q),
                        xf[b, 5 * a:5 * a + 5].rearrange(
                            "c (p q) -> p c q", p=P))
                    ch = lambda c: in_t[:, c * Q:(c + 1) * Q]
                    conf, w_ch, h_ch = ch(4), ch(2), ch(3)
                    out_t = opool.tile([P, NCH * Q], f32, tag="out")
                    o3 = out_t.rearrange("p (q f) -> p q f", f=NCH)
                    t3 = spool.tile([P, 3 * Q], f32, tag="t3")

                    # --- ACT (all in the exp_and_others table set) ---
                    # tanh(cx/2), tanh(cy/2) in one op; tanh(conf/2) separate
                    nc.scalar.activation(t3[:, Q:3 * Q], in_t[:, 0:2 * Q],
                                         AF.Tanh, scale=0.5)
                    nc.scalar.activation(t3[:, 0:Q], conf, AF.Tanh, scale=0.5)
                    nc.scalar.activation(o3[:, :, 2], w_ch, AF.Exp,
                                         bias=gt[:, 2 * Q + 2 * a:2 * Q + 2 * a + 1])
                    nc.scalar.activation(o3[:, :, 3], h_ch, AF.Exp,
                                         bias=gt[:, 2 * Q + 2 * a + 1:2 * Q + 2 * a + 2])
                    t_c = t3[:, 0:Q]
                    t_x = t3[:, Q:2 * Q]
                    t_y = t3[:, 2 * Q:3 * Q]

                    # --- DVE ---
                    # px = 4*tanh(cx/2) + (8*gx+4);  py likewise with gy
                    nc.vector.scalar_tensor_tensor(
                        o3[:, :, 0], t_x, 4.0, gx, op0=OP.mult, op1=OP.add)
                    nc.vector.scalar_tensor_tensor(
                        o3[:, :, 1], t_y, 4.0, gy, op0=OP.mult, op1=OP.add)
                    # pred_conf = 0.5*tanh(conf/2) + 0.5
                    nc.vector.tensor_scalar(
                        o3[:, :, 4], t_c, 0.5, 0.5, OP.mult, OP.add)
                    # count(conf > 0); op1 is the reduce op with accum_out
                    ind = spool.tile([P, Q], f32, tag="ind")
                    nc.vector.tensor_scalar(
                        ind[:], conf, 0.0, None, OP.is_gt, OP.add,
                        accum_out=acc_t[:, a:a + 1])

                    # --- 1.31 MB output DMA per (b, a) via SWDGE ---
                    nc.gpsimd.dma_start(
                        out[b, a * G * G:(a + 1) * G * G, :]
                            .rearrange("(p q) f -> p (q f)", p=P),
                        out_t[:])
                nc.gpsimd.dma_start(accd[b], acc_t[:])

    _split_multiwaits(nc)
    return nc


def _make_grids():
    g = np.empty((P, 2 * Q + 2 * A), np.float32)
    q = np.arange(Q)
    p = np.arange(P)[:, None]
    g[:, 0:Q] = np.broadcast_to(8.0 * (q % 256) + 4.0, (P, Q))
    g[:, Q:2 * Q] = 8.0 * (2 * p + q // 256) + 4.0
    for a in range(A):
        g[:, 2 * Q + 2 * a] = math.log(float(ANCHORS[a][0]))
        g[:, 2 * Q + 2 * a + 1] = math.log(float(ANCHORS[a][1]))
    return g


def _host_assemble(x, targets, S_cnt):
    """Sparse (O(T)) part of the reference + scalar assembly.

    S_cnt: device count of conf>0 (== sum(pred_conf>0.5)), float64.
    """
    import jax.numpy as jnp

    xf = x.reshape(B, A, NCH, G, G)
    anchors_g = (ANCHORS / np.float32(STRIDE)).astype(np.float32)  # grid units

    # ---- build_targets mirrored in f32 (verified equal to device) ----
    tg = targets.astype(np.float32)
    tbox = tg[:, 2:6] * np.float32(G)
    gxy, gwh = tbox[:, :2], tbox[:, 2:]
    aw, ah = anchors_g[:, 0:1], anchors_g[:, 1:2]
    gw, gh = gwh[None, :, 0], gwh[None, :, 1]
    inter = np.minimum(aw, gw) * np.minimum(ah, gh)
    union = aw * ah + gw * gh - inter
    ious = inter / (union + np.float32(EPS))                   # [A,T]
    best_n = np.argmax(ious, axis=0).astype(np.int32)
    bidx = tg[:, 0].astype(np.int32)
    gi = np.clip(np.floor(gxy[:, 0]), 0, G - 1).astype(np.int32)
    gj = np.clip(np.floor(gxy[:, 1]), 0, G - 1).astype(np.int32)

    # Scatter winner per obj cell.  jax .at[].set with duplicate indices is
    # order-undefined and on this backend the winner is neither uniformly
    # the first nor the last update -- resolve duplicated cells with one
    # real XLA index-scatter (same index arrays as the reference's
    # scatters, so the same internal update order picks the same winner).
    cell_targets = {}
    for t in range(T):
        cell_targets.setdefault(
            (int(bidx[t]), int(best_n[t]), int(gj[t]), int(gi[t])), []).append(t)
    win = {c: ts[0] for c, ts in cell_targets.items()}
    dup_cells = [c for c, ts in cell_targets.items() if len(ts) > 1]
    if dup_cells:
        scat = jnp.zeros((B, A, G, G), jnp.float32).at[
            jnp.asarray(bidx), jnp.asarray(best_n),
            jnp.asarray(gj), jnp.asarray(gi)].set(
            jnp.arange(T, dtype=jnp.float32))
        scat = np.asarray(scat)
        for c in dup_cells:
            win[c] = int(scat[c])

    obj_cells = list(win.keys())
    n_obj = len(obj_cells)

    # no_obj_mask on this backend: scatter-min zeroes every cell NOT touched
    # by the (t, a) index set; touched cells get correct min semantics.
    # => ones exactly at touched cells that are neither obj cells nor
    # killed by the ignore threshold.
    keep = (ious.T <= np.float32(IGNORE_THRES))                # [T,A] bool
    obj_set = set(obj_cells)
    noobj_state = {}
    for t in range(T):
        bb, jj, ii = int(bidx[t]), int(gj[t]), int(gi[t])
        for a in range(A):
            c = (bb, a, jj, ii)
            v = noobj_state.get(c, c not in obj_set)
            noobj_state[c] = v and bool(keep[t, a])
    noobj_ones = [c for c, v in noobj_state.items() if v]
    n_noobj = len(noobj_ones)

    # ---- gathers ----
    oc = np.array(obj_cells, np.int64)
    tsel = np.array([win[c] for c in obj_cells], np.int64)
    cx_r = xf[oc[:, 0], oc[:, 1], 0, oc[:, 2], oc[:, 3]]
    cy_r = xf[oc[:, 0], oc[:, 1], 1, oc[:, 2], oc[:, 3]]
    w_r = xf[oc[:, 0], oc[:, 1], 2, oc[:, 2], oc[:, 3]]
    h_r = xf[oc[:, 0], oc[:, 1], 3, oc[:, 2], oc[:, 3]]
    cf_r = xf[oc[:, 0], oc[:, 1], 4, oc[:, 2], oc[:, 3]]
    nz = np.array(noobj_ones, np.int64) if n_noobj else np.zeros((0, 4), np.int64)
    cf_nz = xf[nz[:, 0], nz[:, 1], 4, nz[:, 2], nz[:, 3]]

    # ---- unary transcendentals via jnp (match XLA lowering bitwise) ----
    n1, n2 = len(cx_r), len(cf_nz)
    import jax
    sig_in = np.concatenate([cx_r, cy_r, cf_r, cf_nz]).astype(np.float32)
    sig = np.asarray(jax.nn.sigmoid(jnp.asarray(sig_in)))
    sx, sy = sig[0:n1], sig[n1:2 * n1]
    pc_obj, pc_nz = sig[2 * n1:3 * n1], sig[3 * n1:3 * n1 + n2]

    exp_in = np.concatenate([w_r, h_r]).astype(np.float32)
    ex = np.asarray(jnp.exp(jnp.asarray(exp_in)))
    ew, eh = ex[0:n1], ex[n1:2 * n1]

    # log args, all f32 exactly as the reference forms them
    aw_t = anchors_g[oc[:, 1], 0]
    ah_t = anchors_g[oc[:, 1], 1]
    gwa = (gwh[tsel, 0] / aw_t + np.float32(EPS)).astype(np.float32)
    gha = (gwh[tsel, 1] / ah_t + np.float32(EPS)).astype(np.float32)
    log_in = np.concatenate([
        (pc_obj + np.float32(BCE_EPS)).astype(np.float32),
        (np.float32(1.0) - pc_nz + np.float32(BCE_EPS)).astype(np.float32),
        gwa, gha]).astype(np.float32)
    lg = np.asarray(jnp.log(jnp.asarray(log_in)))
    log_pc = lg[0:n1]
    log_1mpc = lg[n1:n1 + n2]
    tw = lg[n1 + n2:n1 + n2 + n1]
    th = lg[n1 + n2 + n1:]

    # ---- losses (f64 accumulation of f32 cell values) ----
    tx = (gxy[tsel, 0] - oc[:, 3].astype(np.float32)).astype(np.float32)
    ty = (gxy[tsel, 1] - oc[:, 2].astype(np.float32)).astype(np.float32)
    denom = max(float(n_obj), 1.0)
    loss_x = float(((sx - tx).astype(np.float32) ** 2).astype(np.float64).sum()) / denom
    loss_y = float(((sy - ty).astype(np.float32) ** 2).astype(np.float64).sum()) / denom
    loss_w = float(((w_r - tw).astype(np.float32) ** 2).astype(np.float64).sum()) / denom
    loss_h = float(((h_r - th).astype(np.float32) ** 2).astype(np.float64).sum()) / denom
    loss_conf_obj = float((-log_pc).astype(np.float64).sum()) / denom
    loss_conf_no_obj = (float((-log_1mpc).astype(np.float64).sum())
                        / max(float(n_noobj), 1.0))
    conf_obj = float(pc_obj.astype(np.float64).sum()) / denom
    conf_no_obj = (float(pc_nz.astype(np.float64).sum())
                   / max(float(n_noobj), 1.0))

    # ---- iou of predicted boxes vs target boxes at obj cells (f32) ----
    px = (oc[:, 3].astype(np.float32) + sx).astype(np.float32)
    py = (oc[:, 2].astype(np.float32) + sy).astype(np.float32)
    pw = (ew * aw_t).astype(np.float32)
    ph = (eh * ah_t).astype(np.float32)
    bx, by = tbox[tsel, 0], tbox[tsel, 1]
    bw, bh = tbox[tsel, 2], tbox[tsel, 3]
    iw = np.clip(np.minimum(px + pw / 2, bx + bw / 2)
                 - np.maximum(px - pw / 2, bx - bw / 2), 0.0, None)
    ih = np.clip(np.minimum(py + ph / 2, by + bh / 2)
                 - np.maximum(py - ph / 2, by - bh / 2), 0.0, None)
    inter_a = iw * ih
    a1 = ((px + pw / 2) - (px - pw / 2)) * ((py + ph / 2) - (py - ph / 2))
    a2 = ((bx + bw / 2) - (bx - bw / 2)) * ((by + bh / 2) - (by - bh / 2))
    iou = inter_a / (a1 + a2 - inter_a + np.float32(EPS))
    detected = (pc_obj > 0.5).astype(np.float64)
    iou50_det = float(((iou > 0.5).astype(np.float64) * detected).sum())
    iou75_det = float(((iou > 0.75).astype(np.float64) * detected).sum())

    loss_bbox = loss_x + loss_y + loss_w + loss_h
    loss_conf = OBJ_SCALE * loss_conf_obj + NO_OBJ_SCALE * loss_conf_no_obj
    loss_layer = loss_bbox + loss_conf
    precision = iou50_det / (S_cnt + EPS)
    recall50 = iou50_det / (n_obj + EPS)
    recall75 = iou75_det / (n_obj + EPS)

    metrics = np.array([
        loss_x, loss_y, loss_w, loss_h, loss_bbox, loss_conf, loss_layer,
        conf_obj, conf_no_obj, precision, recall50, recall75,
    ], np.float32)
    return np.float32(loss_layer), metrics


def kernel(x, targets, _trace=False):
    global _PROG
    from concourse.bass_utils import run_bass_kernel_spmd

    x = np.ascontiguousarray(x, np.float32)
    targets = np.ascontiguousarray(targets, np.float32)

    if _PROG is None:
        _PROG = _build_program()
    grids = _make_grids()
    in_maps = [
        {"x": x[c * B_LOC:(c + 1) * B_LOC], "grids": grids}
        for c in range(N_CORES)
    ]
    res = run_bass_kernel_spmd(
        _PROG, in_maps, core_ids=list(range(N_CORES)), trace=_trace)
    outs = [res.results[c]["out"] for c in range(N_CORES)]
    accs = [res.results[c]["acc"].astype(np.float64) for c in range(N_CORES)]
    output = np.concatenate(outs, axis=0)
    acc = np.stack(accs)                                        # [8,4,128,3]
    S_cnt = acc.sum()

    loss_layer, metrics = _host_assemble(x, targets, S_cnt)
    if _trace:
        kernel._last_exec_ns = res.exec_time_ns
        kernel._last_results = res
    return output, loss_layer, metrics


# revision 22
# speedup vs baseline: 1.1489x; 1.1489x over previous
"""Trainium2 Bass kernel for nn_BoxRegression_42125039239326 (YOLOv3 loss).

Contract: kernel(**inputs) takes FULL inputs (x:[32,15,256,256] f32,
targets:[4096,6] f32) and returns the full reference output tuple
(output:[32,196608,5] f32, loss_layer: f32 scalar, metrics:[12] f32).

Strategy (data-parallel over batch, 8 cores x 4 batches):
  Device (per core): the memory-bound full-grid work --
    - elementwise transform of x into `output` (sigmoid/exp + grid offsets,
      interleaved to channel-last), using one ACT table set (exp_and_others):
      sigmoid(c) = 0.5 + 0.5*tanh(c/2); exp(w)*anchor = exp(w + ln(anchor)).
    - full-grid reduction: count(conf>0) (== count(pred_conf>0.5), the
      precision metric's denominator).
  Host: O(T)=4096-sparse per-target work (build_targets scatter logic and
    the masked means, which are all sparse:

    NOTE on this XLA backend (neuron), `.at[...].min(keep)` scatter-min
    ZEROES all un-touched cells (verified empirically), so the reference's
    no_obj_mask is nonzero only at target-touched cells (~8k) -- the
    "no-obj" losses are therefore sparse sums over those cells, not
    full-grid reductions.  The unary transcendentals for the sparse values
    go through jnp so they match the reference's XLA lowerings bitwise.)
"""

import math

import numpy as np

# ---------------------------------------------------------------------------
# constants (hardcoded per problem spec)
B, A, G, T = 32, 3, 256, 4096
N_CORES = 8
B_LOC = B // N_CORES
P, Q = 128, 512          # SBUF tile: 128 partitions x 512 cells (2 grid rows)
NCH = 5
ANCHORS = np.array([[116.0, 90.0], [156.0, 198.0], [373.0, 326.0]], np.float32)
IMAGE_SIZE = 2048.0
STRIDE = IMAGE_SIZE / G          # 8.0
IGNORE_THRES = 0.5
OBJ_SCALE = 100.0
NO_OBJ_SCALE = 1.0
EPS = 1e-16
BCE_EPS = 1e-12
N_ALL = B * A * G * G


def _split_multiwaits(nc):
    """Walrus in this container allows 1 sem-wait per instruction; Tile's
    tail drain can carry several.  Splice extra waits onto preceding NoOps."""
    import concourse.mybir as mybir
    import bass_rust

    for f in nc.m.functions:
        for bb in f.blocks:
            new_insts = []
            for ins in bb.instructions:
                si = ins.sync_info
                if si is not None and si.on_wait is not None and len(si.on_wait) > 1:
                    waits = list(si.on_wait)
                    for i, w in enumerate(waits[:-1]):
                        nop = mybir.InstNoOp(name=f"{ins.name}-wsplit{i}")
                        nop.engine = ins.engine
                        nop.sync_info = bass_rust.SyncInfo(on_wait=[w], on_update=[])
                        new_insts.append(nop)
                    ins.sync_info = bass_rust.SyncInfo(
                        on_wait=[waits[-1]], on_update=list(si.on_update or [])
                    )
                new_insts.append(ins)
            bb.instructions = new_insts


_PROG = None


def _build_program():
    import concourse.bass as bass
    import concourse.mybir as mybir
    from concourse.tile import TileContext

    AF = mybir.ActivationFunctionType
    OP = mybir.AluOpType
    f32 = mybir.dt.float32

    nc = bass.Bass()
    x = nc.dram_tensor("x", [B_LOC, A * NCH, G, G], f32, kind="ExternalInput")
    grids = nc.dram_tensor("grids", [P, 2 * Q + 2 * A], f32, kind="ExternalInput")
    out = nc.dram_tensor("out", [B_LOC, A * G * G, NCH], f32, kind="ExternalOutput")
    accd = nc.dram_tensor("acc", [B_LOC, P, A], f32, kind="ExternalOutput")

    xf = x.rearrange("b c h w -> b c (h w)")

    with TileContext(nc) as tc:
        with tc.tile_pool(name="const", bufs=1) as cpool, \
             tc.tile_pool(name="inp", bufs=4) as ipool, \
             tc.tile_pool(name="outp", bufs=4) as opool, \
             tc.tile_pool(name="scr", bufs=3) as spool:
            gt = cpool.tile([P, 2 * Q + 2 * A], f32)
            nc.gpsimd.dma_start(gt[:], grids[:])
            gx = gt[:, 0:Q]          # 8*gx + 4
            gy = gt[:, Q:2 * Q]      # 8*gy + 4

            for b in range(B_LOC):
                acc_t = spool.tile([P, A], f32, tag="acc")
                for a in range(A):
                    # one 1.25 MB input DMA per (b, a) on the SP HWDGE ring;
                    # outputs ride SWDGE (gpsimd) so reads and writes stream
                    # concurrently on the 16 SDMA engines.
                    in_t = ipool.tile([P, NCH * Q], f32, tag="in")
                    # batch 0 inputs ride SWDGE: its issue path comes up
                    # ~4.5us earlier than the SP HWDGE ring at NEFF start
                    in_eng = nc.gpsimd if b == 0 else nc.sync
                    in_eng.dma_start(
                        in_t[:].rearrange("p (c q) -> p c q", q=Q),
                        xf[b, 5 * a:5 * a + 5].rearrange(
                            "c (p q) -> p c q", p=P))
                    ch = lambda c: in_t[:, c * Q:(c + 1) * Q]
                    conf, w_ch, h_ch = ch(4), ch(2), ch(3)
                    out_t = opool.tile([P, NCH * Q], f32, tag="out")
                    o3 = out_t.rearrange("p (q f) -> p q f", f=NCH)
                    t3 = spool.tile([P, 3 * Q], f32, tag="t3")

                    # --- ACT (all in the exp_and_others table set) ---
                    # tanh(cx/2), tanh(cy/2) in one op; tanh(conf/2) separate
                    nc.scalar.activation(t3[:, Q:3 * Q], in_t[:, 0:2 * Q],
                                         AF.Tanh, scale=0.5)
                    nc.scalar.activation(t3[:, 0:Q], conf, AF.Tanh, scale=0.5)
                    nc.scalar.activation(o3[:, :, 2], w_ch, AF.Exp,
                                         bias=gt[:, 2 * Q + 2 * a:2 * Q + 2 * a + 1])
                    nc.scalar.activation(o3[:, :, 3], h_ch, AF.Exp,
                                         bias=gt[:, 2 * Q + 2 * a + 1:2 * Q + 2 * a + 2])
                    t_c = t3[:, 0:Q]
                    t_x = t3[:, Q:2 * Q]
                    t_y = t3[:, 2 * Q:3 * Q]

                    # --- DVE ---
                    # px = 4*tanh(cx/2) + (8*gx+4);  py likewise with gy
                    nc.vector.scalar_tensor_tensor(
                        o3[:, :, 0], t_x, 4.0, gx, op0=OP.mult, op1=OP.add)
                    nc.vector.scalar_tensor_tensor(
                        o3[:, :, 1], t_y, 4.0, gy, op0=OP.mult, op1=OP.add)
                    # pred_conf = 0.5*tanh(conf/2) + 0.5
                    nc.vector.tensor_scalar(
                        o3[:, :, 4], t_c, 0.5, 0.5, OP.mult, OP.add)
                    # count(conf > 0); op1 is the reduce op with accum_out
                    ind = spool.tile([P, Q], f32, tag="ind")
                    nc.vector.tensor_scalar(
                        ind[:], conf, 0.0, None, OP.is_gt, OP.add,
                        accum_out=acc_t[:, a:a + 1])

                    # --- 1.31 MB output DMA per (b, a) via SWDGE ---
                    # (the very last one is split so the final HBM-write
                    # completion covers a smaller in-flight window)
                    oba = out[b, a * G * G:(a + 1) * G * G, :] \
                        .rearrange("(p q) f -> p (q f)", p=P)
                    if b == B_LOC - 1 and a == A - 1:
                        nc.gpsimd.dma_start(
                            oba[:, 0:NCH * Q // 2], out_t[:, 0:NCH * Q // 2])
                        nc.gpsimd.dma_start(
                            oba[:, NCH * Q // 2:], out_t[:, NCH * Q // 2:])
                    else:
                        nc.gpsimd.dma_start(oba, out_t[:])
                nc.gpsimd.dma_start(accd[b], acc_t[:])

    _split_multiwaits(nc)
    return nc


def _make_grids():
    g = np.empty((P, 2 * Q + 2 * A), np.float32)
    q = np.arange(Q)
    p = np.arange(P)[:, None]
    g[:, 0:Q] = np.broadcast_to(8.0 * (q % 256) + 4.0, (P, Q))
    g[:, Q:2 * Q] = 8.0 * (2 * p + q // 256) + 4.0
    for a in range(A):
        g[:, 2 * Q + 2 * a] = math.log(float(ANCHORS[a][0]))
        g[:, 2 * Q + 2 * a + 1] = math.log(float(ANCHORS[a][1]))
    return g


def _host_assemble(x, targets, S_cnt):
    """Sparse (O(T)) part of the reference + scalar assembly.

    S_cnt: device count of conf>0 (== sum(pred_conf>0.5)), float64.
    """
    import jax.numpy as jnp

    xf = x.reshape(B, A, NCH, G, G)
    anchors_g = (ANCHORS / np.float32(STRIDE)).astype(np.float32)  # grid units

    # ---- build_targets mirrored in f32 (verified equal to device) ----
    tg = targets.astype(np.float32)
    tbox = tg[:, 2:6] * np.float32(G)
    gxy, gwh = tbox[:, :2], tbox[:, 2:]
    aw, ah = anchors_g[:, 0:1], anchors_g[:, 1:2]
    gw, gh = gwh[None, :, 0], gwh[None, :, 1]
    inter = np.minimum(aw, gw) * np.minimum(ah, gh)
    union = aw * ah + gw * gh - inter
    ious = inter / (union + np.float32(EPS))                   # [A,T]
    best_n = np.argmax(ious, axis=0).astype(np.int32)
    bidx = tg[:, 0].astype(np.int32)
    gi = np.clip(np.floor(gxy[:, 0]), 0, G - 1).astype(np.int32)
    gj = np.clip(np.floor(gxy[:, 1]), 0, G - 1).astype(np.int32)

    # Scatter winner per obj cell.  jax .at[].set with duplicate indices is
    # order-undefined and on this backend the winner is neither uniformly
    # the first nor the last update -- resolve duplicated cells with one
    # real XLA index-scatter (same index arrays as the reference's
    # scatters, so the same internal update order picks the same winner).
    cell_targets = {}
    for t in range(T):
        cell_targets.setdefault(
            (int(bidx[t]), int(best_n[t]), int(gj[t]), int(gi[t])), []).append(t)
    win = {c: ts[0] for c, ts in cell_targets.items()}
    dup_cells = [c for c, ts in cell_targets.items() if len(ts) > 1]
    if dup_cells:
        scat = jnp.zeros((B, A, G, G), jnp.float32).at[
            jnp.asarray(bidx), jnp.asarray(best_n),
            jnp.asarray(gj), jnp.asarray(gi)].set(
            jnp.arange(T, dtype=jnp.float32))
        scat = np.asarray(scat)
        for c in dup_cells:
            win[c] = int(scat[c])

    obj_cells = list(win.keys())
    n_obj = len(obj_cells)

    # no_obj_mask on this backend: scatter-min zeroes every cell NOT touched
    # by the (t, a) index set; touched cells get correct min semantics.
    # => ones exactly at touched cells that are neither obj cells nor
    # killed by the ignore threshold.
    keep = (ious.T <= np.float32(IGNORE_THRES))                # [T,A] bool
    obj_set = set(obj_cells)
    noobj_state = {}
    for t in range(T):
        bb, jj, ii = int(bidx[t]), int(gj[t]), int(gi[t])
        for a in range(A):
            c = (bb, a, jj, ii)
            v = noobj_state.get(c, c not in obj_set)
            noobj_state[c] = v and bool(keep[t, a])
    noobj_ones = [c for c, v in noobj_state.items() if v]
    n_noobj = len(noobj_ones)

    # ---- gathers ----
    oc = np.array(obj_cells, np.int64)
    tsel = np.array([win[c] for c in obj_cells], np.int64)
    cx_r = xf[oc[:, 0], oc[:, 1], 0, oc[:, 2], oc[:, 3]]
    cy_r = xf[oc[:, 0], oc[:, 1], 1, oc[:, 2], oc[:, 3]]
    w_r = xf[oc[:, 0], oc[:, 1], 2, oc[:, 2], oc[:, 3]]
    h_r = xf[oc[:, 0], oc[:, 1], 3, oc[:, 2], oc[:, 3]]
    cf_r = xf[oc[:, 0], oc[:, 1], 4, oc[:, 2], oc[:, 3]]
    nz = np.array(noobj_ones, np.int64) if n_noobj else np.zeros((0, 4), np.int64)
    cf_nz = xf[nz[:, 0], nz[:, 1], 4, nz[:, 2], nz[:, 3]]

    # ---- unary transcendentals via jnp (match XLA lowering bitwise) ----
    n1, n2 = len(cx_r), len(cf_nz)
    import jax
    sig_in = np.concatenate([cx_r, cy_r, cf_r, cf_nz]).astype(np.float32)
    sig = np.asarray(jax.nn.sigmoid(jnp.asarray(sig_in)))
    sx, sy = sig[0:n1], sig[n1:2 * n1]
    pc_obj, pc_nz = sig[2 * n1:3 * n1], sig[3 * n1:3 * n1 + n2]

    exp_in = np.concatenate([w_r, h_r]).astype(np.float32)
    ex = np.asarray(jnp.exp(jnp.asarray(exp_in)))
    ew, eh = ex[0:n1], ex[n1:2 * n1]

    # log args, all f32 exactly as the reference forms them
    aw_t = anchors_g[oc[:, 1], 0]
    ah_t = anchors_g[oc[:, 1], 1]
    gwa = (gwh[tsel, 0] / aw_t + np.float32(EPS)).astype(np.float32)
    gha = (gwh[tsel, 1] / ah_t + np.float32(EPS)).astype(np.float32)
    log_in = np.concatenate([
        (pc_obj + np.float32(BCE_EPS)).astype(np.float32),
        (np.float32(1.0) - pc_nz + np.float32(BCE_EPS)).astype(np.float32),
        gwa, gha]).astype(np.float32)
    lg = np.asarray(jnp.log(jnp.asarray(log_in)))
    log_pc = lg[0:n1]
    log_1mpc = lg[n1:n1 + n2]
    tw = lg[n1 + n2:n1 + n2 + n1]
    th = lg[n1 + n2 + n1:]

    # ---- losses (f64 accumulation of f32 cell values) ----
    tx = (gxy[tsel, 0] - oc[:, 3].astype(np.float32)).astype(np.float32)
    ty = (gxy[tsel, 1] - oc[:, 2].astype(np.float32)).astype(np.float32)
    denom = max(float(n_obj), 1.0)
    loss_x = float(((sx - tx).astype(np.float32) ** 2).astype(np.float64).sum()) / denom
    loss_y = float(((sy - ty).astype(np.float32) ** 2).astype(np.float64).sum()) / denom
    loss_w = float(((w_r - tw).astype(np.float32) ** 2).astype(np.float64).sum()) / denom
    loss_h = float(((h_r - th).astype(np.float32) ** 2).astype(np.float64).sum()) / denom
    loss_conf_obj = float((-log_pc).astype(np.float64).sum()) / denom
    loss_conf_no_obj = (float((-log_1mpc).astype(np.float64).sum())
                        / max(float(n_noobj), 1.0))
    conf_obj = float(pc_obj.astype(np.float64).sum()) / denom
    conf_no_obj = (float(pc_nz.astype(np.float64).sum())
                   / max(float(n_noobj), 1.0))

    # ---- iou of predicted boxes vs target boxes at obj cells (f32) ----
    px = (oc[:, 3].astype(np.float32) + sx).astype(np.float32)
    py = (oc[:, 2].astype(np.float32) + sy).astype(np.float32)
    pw = (ew * aw_t).astype(np.float32)
    ph = (eh * ah_t).astype(np.float32)
    bx, by = tbox[tsel, 0], tbox[tsel, 1]
    bw, bh = tbox[tsel, 2], tbox[tsel, 3]
    iw = np.clip(np.minimum(px + pw / 2, bx + bw / 2)
                 - np.maximum(px - pw / 2, bx - bw / 2), 0.0, None)
    ih = np.clip(np.minimum(py + ph / 2, by + bh / 2)
                 - np.maximum(py - ph / 2, by - bh / 2), 0.0, None)
    inter_a = iw * ih
    a1 = ((px + pw / 2) - (px - pw / 2)) * ((py + ph / 2) - (py - ph / 2))
    a2 = ((bx + bw / 2) - (bx - bw / 2)) * ((by + bh / 2) - (by - bh / 2))
    iou = inter_a / (a1 + a2 - inter_a + np.float32(EPS))
    detected = (pc_obj > 0.5).astype(np.float64)
    iou50_det = float(((iou > 0.5).astype(np.float64) * detected).sum())
    iou75_det = float(((iou > 0.75).astype(np.float64) * detected).sum())

    loss_bbox = loss_x + loss_y + loss_w + loss_h
    loss_conf = OBJ_SCALE * loss_conf_obj + NO_OBJ_SCALE * loss_conf_no_obj
    loss_layer = loss_bbox + loss_conf
    precision = iou50_det / (S_cnt + EPS)
    recall50 = iou50_det / (n_obj + EPS)
    recall75 = iou75_det / (n_obj + EPS)

    metrics = np.array([
        loss_x, loss_y, loss_w, loss_h, loss_bbox, loss_conf, loss_layer,
        conf_obj, conf_no_obj, precision, recall50, recall75,
    ], np.float32)
    return np.float32(loss_layer), metrics


def kernel(x, targets, _trace=False):
    global _PROG
    from concourse.bass_utils import run_bass_kernel_spmd

    x = np.ascontiguousarray(x, np.float32)
    targets = np.ascontiguousarray(targets, np.float32)

    if _PROG is None:
        _PROG = _build_program()
    grids = _make_grids()
    in_maps = [
        {"x": x[c * B_LOC:(c + 1) * B_LOC], "grids": grids}
        for c in range(N_CORES)
    ]
    res = run_bass_kernel_spmd(
        _PROG, in_maps, core_ids=list(range(N_CORES)), trace=_trace)
    outs = [res.results[c]["out"] for c in range(N_CORES)]
    accs = [res.results[c]["acc"].astype(np.float64) for c in range(N_CORES)]
    output = np.concatenate(outs, axis=0)
    acc = np.stack(accs)                                        # [8,4,128,3]
    S_cnt = acc.sum()

    loss_layer, metrics = _host_assemble(x, targets, S_cnt)
    if _trace:
        kernel._last_exec_ns = res.exec_time_ns
        kernel._last_results = res
    return output, loss_layer, metrics


# revision 28
# speedup vs baseline: 1.1509x; 1.0017x over previous
"""Trainium2 Bass kernel for nn_BoxRegression_42125039239326 (YOLOv3 loss).

Contract: kernel(**inputs) takes FULL inputs (x:[32,15,256,256] f32,
targets:[4096,6] f32) and returns the full reference output tuple
(output:[32,196608,5] f32, loss_layer: f32 scalar, metrics:[12] f32).

Strategy (data-parallel over batch, 8 cores x 4 batches):
  Device (per core): the memory-bound full-grid work --
    - elementwise transform of x into `output` (sigmoid/exp + grid offsets,
      interleaved to channel-last), using one ACT table set (exp_and_others):
      sigmoid(c) = 0.5 + 0.5*tanh(c/2); exp(w)*anchor = exp(w + ln(anchor)).
    - full-grid reduction: count(conf>0) (== count(pred_conf>0.5), the
      precision metric's denominator).
  Host: O(T)=4096-sparse per-target work (build_targets scatter logic and
    the masked means, which are all sparse:

    NOTE on this XLA backend (neuron), `.at[...].min(keep)` scatter-min
    ZEROES all un-touched cells (verified empirically), so the reference's
    no_obj_mask is nonzero only at target-touched cells (~8k) -- the
    "no-obj" losses are therefore sparse sums over those cells, not
    full-grid reductions.  The unary transcendentals for the sparse values
    go through jnp so they match the reference's XLA lowerings bitwise.)
"""

import math

import numpy as np

# ---------------------------------------------------------------------------
# constants (hardcoded per problem spec)
B, A, G, T = 32, 3, 256, 4096
N_CORES = 8
B_LOC = B // N_CORES
P, Q = 128, 512          # SBUF tile: 128 partitions x 512 cells (2 grid rows)
NCH = 5
ANCHORS = np.array([[116.0, 90.0], [156.0, 198.0], [373.0, 326.0]], np.float32)
IMAGE_SIZE = 2048.0
STRIDE = IMAGE_SIZE / G          # 8.0
IGNORE_THRES = 0.5
OBJ_SCALE = 100.0
NO_OBJ_SCALE = 1.0
EPS = 1e-16
BCE_EPS = 1e-12
N_ALL = B * A * G * G


def _split_multiwaits(nc):
    """Walrus in this container allows 1 sem-wait per instruction; Tile's
    tail drain can carry several.  Splice extra waits onto preceding NoOps."""
    import concourse.mybir as mybir
    import bass_rust

    for f in nc.m.functions:
        for bb in f.blocks:
            new_insts = []
            for ins in bb.instructions:
                si = ins.sync_info
                if si is not None and si.on_wait is not None and len(si.on_wait) > 1:
                    waits = list(si.on_wait)
                    for i, w in enumerate(waits[:-1]):
                        nop = mybir.InstNoOp(name=f"{ins.name}-wsplit{i}")
                        nop.engine = ins.engine
                        nop.sync_info = bass_rust.SyncInfo(on_wait=[w], on_update=[])
                        new_insts.append(nop)
                    ins.sync_info = bass_rust.SyncInfo(
                        on_wait=[waits[-1]], on_update=list(si.on_update or [])
                    )
                new_insts.append(ins)
            bb.instructions = new_insts


_PROG = None


def _build_program():
    import concourse.bass as bass
    import concourse.mybir as mybir
    from concourse.tile import TileContext

    AF = mybir.ActivationFunctionType
    OP = mybir.AluOpType
    f32 = mybir.dt.float32

    nc = bass.Bass()
    x = nc.dram_tensor("x", [B_LOC, A * NCH, G, G], f32, kind="ExternalInput")
    # grids layout: [gx8p4 | gy8p4 | ln-anchor pairs | zeros column]
    grids = nc.dram_tensor("grids", [P, 2 * Q + 2 * A + 1], f32,
                           kind="ExternalInput")
    out = nc.dram_tensor("out", [B_LOC, A * G * G, NCH], f32, kind="ExternalOutput")
    accd = nc.dram_tensor("acc", [B_LOC, P, A], f32, kind="ExternalOutput")

    xf = x.rearrange("b c h w -> b c (h w)")

    with TileContext(nc) as tc:
        with tc.tile_pool(name="const", bufs=1) as cpool, \
             tc.tile_pool(name="inp", bufs=4) as ipool, \
             tc.tile_pool(name="outp", bufs=4) as opool, \
             tc.tile_pool(name="scr", bufs=3) as spool:
            gt = cpool.tile([P, 2 * Q + 2 * A + 1], f32)
            nc.gpsimd.dma_start(gt[:], grids[:])
            gx = gt[:, 0:Q]          # 8*gx + 4
            gy = gt[:, Q:2 * Q]      # 8*gy + 4
            zcol = gt[:, 2 * Q + 2 * A:2 * Q + 2 * A + 1]

            for b in range(B_LOC):
                acc_t = spool.tile([P, A], f32, tag="acc")
                for a in range(A):
                    # one 1.25 MB input DMA per (b, a) on the SP HWDGE ring;
                    # outputs ride SWDGE (gpsimd) so reads and writes stream
                    # concurrently on the 16 SDMA engines.
                    in_t = ipool.tile([P, NCH * Q], f32, tag="in")
                    # batch 0 inputs ride SWDGE: its issue path comes up
                    # ~4.5us earlier than the SP HWDGE ring at NEFF start
                    in_eng = nc.gpsimd if b == 0 else nc.sync
                    in_eng.dma_start(
                        in_t[:].rearrange("p (c q) -> p c q", q=Q),
                        xf[b, 5 * a:5 * a + 5].rearrange(
                            "c (p q) -> p c q", p=P))
                    ch = lambda c: in_t[:, c * Q:(c + 1) * Q]
                    conf, w_ch, h_ch = ch(4), ch(2), ch(3)
                    out_t = opool.tile([P, NCH * Q], f32, tag="out")
                    o3 = out_t.rearrange("p (q f) -> p q f", f=NCH)
                    t3 = spool.tile([P, 3 * Q], f32, tag="t3")

                    # --- ACT (all in the exp_and_others table set) ---
                    # tanh(cx/2), tanh(cy/2) in one op; tanh(conf/2) separate
                    nc.scalar.activation(t3[:, Q:3 * Q], in_t[:, 0:2 * Q],
                                         AF.Tanh, scale=0.5, bias=zcol)
                    nc.scalar.activation(t3[:, 0:Q], conf, AF.Tanh, scale=0.5,
                                         bias=zcol)
                    nc.scalar.activation(o3[:, :, 2], w_ch, AF.Exp,
                                         bias=gt[:, 2 * Q + 2 * a:2 * Q + 2 * a + 1])
                    nc.scalar.activation(o3[:, :, 3], h_ch, AF.Exp,
                                         bias=gt[:, 2 * Q + 2 * a + 1:2 * Q + 2 * a + 2])
                    t_c = t3[:, 0:Q]
                    t_x = t3[:, Q:2 * Q]
                    t_y = t3[:, 2 * Q:3 * Q]

                    # --- DVE ---
                    # px = 4*tanh(cx/2) + (8*gx+4);  py likewise with gy
                    nc.vector.scalar_tensor_tensor(
                        o3[:, :, 0], t_x, 4.0, gx, op0=OP.mult, op1=OP.add)
                    nc.vector.scalar_tensor_tensor(
                        o3[:, :, 1], t_y, 4.0, gy, op0=OP.mult, op1=OP.add)
                    # pred_conf = 0.5*tanh(conf/2) + 0.5
                    nc.vector.tensor_scalar(
                        o3[:, :, 4], t_c, 0.5, 0.5, OP.mult, OP.add)
                    # count(conf > 0); op1 is the reduce op with accum_out
                    ind = spool.tile([P, Q], f32, tag="ind")
                    nc.vector.tensor_scalar(
                        ind[:], conf, 0.0, None, OP.is_gt, OP.add,
                        accum_out=acc_t[:, a:a + 1])

                    # --- 1.31 MB output DMA per (b, a) via SWDGE ---
                    # (the very last one is split so the final HBM-write
                    # completion covers a smaller in-flight window)
                    oba = out[b, a * G * G:(a + 1) * G * G, :] \
                        .rearrange("(p q) f -> p (q f)", p=P)
                    if b == B_LOC - 1 and a == A - 1:
                        nc.gpsimd.dma_start(
                            oba[:, 0:NCH * Q // 2], out_t[:, 0:NCH * Q // 2])
                        nc.gpsimd.dma_start(
                            oba[:, NCH * Q // 2:], out_t[:, NCH * Q // 2:])
                    else:
                        nc.gpsimd.dma_start(oba, out_t[:])
                nc.gpsimd.dma_start(accd[b], acc_t[:])

    _split_multiwaits(nc)
    _strip_const_preamble(nc)
    return nc


def _strip_const_preamble(nc):
    """Remove the const-AP memsets and the all-engine barrier that guards
    them from the preamble -- nothing in this kernel reads the const APs
    (the Tanh bias comes from the grids tile), and Tile's own semaphores
    cover every real dependency.  Saves ~2-3us of serialized startup."""
    bb = nc.m.functions[0].blocks[0]
    kept = []
    for ins in bb.instructions:
        if ins.opcode == "Memset":
            continue
        if ins.opcode in ("Drain", "EventSemaphore") and (
                ins.name.startswith("I-") or ins.name.startswith("barrier_")):
            # preamble barrier pieces (the const-guard all_engine_barrier)
            continue
        kept.append(ins)
    bb.instructions = kept


def _make_grids():
    g = np.empty((P, 2 * Q + 2 * A + 1), np.float32)
    q = np.arange(Q)
    p = np.arange(P)[:, None]
    g[:, 0:Q] = np.broadcast_to(8.0 * (q % 256) + 4.0, (P, Q))
    g[:, Q:2 * Q] = 8.0 * (2 * p + q // 256) + 4.0
    for a in range(A):
        g[:, 2 * Q + 2 * a] = math.log(float(ANCHORS[a][0]))
        g[:, 2 * Q + 2 * a + 1] = math.log(float(ANCHORS[a][1]))
    g[:, 2 * Q + 2 * A] = 0.0
    return g


def _host_assemble(x, targets, S_cnt):
    """Sparse (O(T)) part of the reference + scalar assembly.

    S_cnt: device count of conf>0 (== sum(pred_conf>0.5)), float64.
    """
    import jax.numpy as jnp

    xf = x.reshape(B, A, NCH, G, G)
    anchors_g = (ANCHORS / np.float32(STRIDE)).astype(np.float32)  # grid units

    # ---- build_targets mirrored in f32 (verified equal to device) ----
    tg = targets.astype(np.float32)
    tbox = tg[:, 2:6] * np.float32(G)
    gxy, gwh = tbox[:, :2], tbox[:, 2:]
    aw, ah = anchors_g[:, 0:1], anchors_g[:, 1:2]
    gw, gh = gwh[None, :, 0], gwh[None, :, 1]
    inter = np.minimum(aw, gw) * np.minimum(ah, gh)
    union = aw * ah + gw * gh - inter
    ious = inter / (union + np.float32(EPS))                   # [A,T]
    best_n = np.argmax(ious, axis=0).astype(np.int32)
    bidx = tg[:, 0].astype(np.int32)
    gi = np.clip(np.floor(gxy[:, 0]), 0, G - 1).astype(np.int32)
    gj = np.clip(np.floor(gxy[:, 1]), 0, G - 1).astype(np.int32)

    # Scatter winner per obj cell.  jax .at[].set with duplicate indices is
    # order-undefined and on this backend the winner is neither uniformly
    # the first nor the last update -- resolve duplicated cells with one
    # real XLA index-scatter (same index arrays as the reference's
    # scatters, so the same internal update order picks the same winner).
    cell_targets = {}
    for t in range(T):
        cell_targets.setdefault(
            (int(bidx[t]), int(best_n[t]), int(gj[t]), int(gi[t])), []).append(t)
    win = {c: ts[0] for c, ts in cell_targets.items()}
    dup_cells = [c for c, ts in cell_targets.items() if len(ts) > 1]
    if dup_cells:
        scat = jnp.zeros((B, A, G, G), jnp.float32).at[
            jnp.asarray(bidx), jnp.asarray(best_n),
            jnp.asarray(gj), jnp.asarray(gi)].set(
            jnp.arange(T, dtype=jnp.float32))
        scat = np.asarray(scat)
        for c in dup_cells:
            win[c] = int(scat[c])

    obj_cells = list(win.keys())
    n_obj = len(obj_cells)

    # no_obj_mask on this backend: scatter-min zeroes every cell NOT touched
    # by the (t, a) index set; touched cells get correct min semantics.
    # => ones exactly at touched cells that are neither obj cells nor
    # killed by the ignore threshold.
    keep = (ious.T <= np.float32(IGNORE_THRES))                # [T,A] bool
    obj_set = set(obj_cells)
    noobj_state = {}
    for t in range(T):
        bb, jj, ii = int(bidx[t]), int(gj[t]), int(gi[t])
        for a in range(A):
            c = (bb, a, jj, ii)
            v = noobj_state.get(c, c not in obj_set)
            noobj_state[c] = v and bool(keep[t, a])
    noobj_ones = [c for c, v in noobj_state.items() if v]
    n_noobj = len(noobj_ones)

    # ---- gathers ----
    oc = np.array(obj_cells, np.int64)
    tsel = np.array([win[c] for c in obj_cells], np.int64)
    cx_r = xf[oc[:, 0], oc[:, 1], 0, oc[:, 2], oc[:, 3]]
    cy_r = xf[oc[:, 0], oc[:, 1], 1, oc[:, 2], oc[:, 3]]
    w_r = xf[oc[:, 0], oc[:, 1], 2, oc[:, 2], oc[:, 3]]
    h_r = xf[oc[:, 0], oc[:, 1], 3, oc[:, 2], oc[:, 3]]
    cf_r = xf[oc[:, 0], oc[:, 1], 4, oc[:, 2], oc[:, 3]]
    nz = np.array(noobj_ones, np.int64) if n_noobj else np.zeros((0, 4), np.int64)
    cf_nz = xf[nz[:, 0], nz[:, 1], 4, nz[:, 2], nz[:, 3]]

    # ---- unary transcendentals via jnp (match XLA lowering bitwise) ----
    n1, n2 = len(cx_r), len(cf_nz)
    import jax
    sig_in = np.concatenate([cx_r, cy_r, cf_r, cf_nz]).astype(np.float32)
    sig = np.asarray(jax.nn.sigmoid(jnp.asarray(sig_in)))
    sx, sy = sig[0:n1], sig[n1:2 * n1]
    pc_obj, pc_nz = sig[2 * n1:3 * n1], sig[3 * n1:3 * n1 + n2]

    exp_in = np.concatenate([w_r, h_r]).astype(np.float32)
    ex = np.asarray(jnp.exp(jnp.asarray(exp_in)))
    ew, eh = ex[0:n1], ex[n1:2 * n1]

    # log args, all f32 exactly as the reference forms them
    aw_t = anchors_g[oc[:, 1], 0]
    ah_t = anchors_g[oc[:, 1], 1]
    gwa = (gwh[tsel, 0] / aw_t + np.float32(EPS)).astype(np.float32)
    gha = (gwh[tsel, 1] / ah_t + np.float32(EPS)).astype(np.float32)
    log_in = np.concatenate([
        (pc_obj + np.float32(BCE_EPS)).astype(np.float32),
        (np.float32(1.0) - pc_nz + np.float32(BCE_EPS)).astype(np.float32),
        gwa, gha]).astype(np.float32)
    lg = np.asarray(jnp.log(jnp.asarray(log_in)))
    log_pc = lg[0:n1]
    log_1mpc = lg[n1:n1 + n2]
    tw = lg[n1 + n2:n1 + n2 + n1]
    th = lg[n1 + n2 + n1:]

    # ---- losses (f64 accumulation of f32 cell values) ----
    tx = (gxy[tsel, 0] - oc[:, 3].astype(np.float32)).astype(np.float32)
    ty = (gxy[tsel, 1] - oc[:, 2].astype(np.float32)).astype(np.float32)
    denom = max(float(n_obj), 1.0)
    loss_x = float(((sx - tx).astype(np.float32) ** 2).astype(np.float64).sum()) / denom
    loss_y = float(((sy - ty).astype(np.float32) ** 2).astype(np.float64).sum()) / denom
    loss_w = float(((w_r - tw).astype(np.float32) ** 2).astype(np.float64).sum()) / denom
    loss_h = float(((h_r - th).astype(np.float32) ** 2).astype(np.float64).sum()) / denom
    loss_conf_obj = float((-log_pc).astype(np.float64).sum()) / denom
    loss_conf_no_obj = (float((-log_1mpc).astype(np.float64).sum())
                        / max(float(n_noobj), 1.0))
    conf_obj = float(pc_obj.astype(np.float64).sum()) / denom
    conf_no_obj = (float(pc_nz.astype(np.float64).sum())
                   / max(float(n_noobj), 1.0))

    # ---- iou of predicted boxes vs target boxes at obj cells (f32) ----
    px = (oc[:, 3].astype(np.float32) + sx).astype(np.float32)
    py = (oc[:, 2].astype(np.float32) + sy).astype(np.float32)
    pw = (ew * aw_t).astype(np.float32)
    ph = (eh * ah_t).astype(np.float32)
    bx, by = tbox[tsel, 0], tbox[tsel, 1]
    bw, bh = tbox[tsel, 2], tbox[tsel, 3]
    iw = np.clip(np.minimum(px + pw / 2, bx + bw / 2)
                 - np.maximum(px - pw / 2, bx - bw / 2), 0.0, None)
    ih = np.clip(np.minimum(py + ph / 2, by + bh / 2)
                 - np.maximum(py - ph / 2, by - bh / 2), 0.0, None)
    inter_a = iw * ih
    a1 = ((px + pw / 2) - (px - pw / 2)) * ((py + ph / 2) - (py - ph / 2))
    a2 = ((bx + bw / 2) - (bx - bw / 2)) * ((by + bh / 2) - (by - bh / 2))
    iou = inter_a / (a1 + a2 - inter_a + np.float32(EPS))
    detected = (pc_obj > 0.5).astype(np.float64)
    iou50_det = float(((iou > 0.5).astype(np.float64) * detected).sum())
    iou75_det = float(((iou > 0.75).astype(np.float64) * detected).sum())

    loss_bbox = loss_x + loss_y + loss_w + loss_h
    loss_conf = OBJ_SCALE * loss_conf_obj + NO_OBJ_SCALE * loss_conf_no_obj
    loss_layer = loss_bbox + loss_conf
    precision = iou50_det / (S_cnt + EPS)
    recall50 = iou50_det / (n_obj + EPS)
    recall75 = iou75_det / (n_obj + EPS)

    metrics = np.array([
        loss_x, loss_y, loss_w, loss_h, loss_bbox, loss_conf, loss_layer,
        conf_obj, conf_no_obj, precision, recall50, recall75,
    ], np.float32)
    return np.float32(loss_layer), metrics


def kernel(x, targets, _trace=False):
    global _PROG
    from concourse.bass_utils import run_bass_kernel_spmd

    x = np.ascontiguousarray(x, np.float32)
    targets = np.ascontiguousarray(targets, np.float32)

    if _PROG is None:
        _PROG = _build_program()
    grids = _make_grids()
    in_maps = [
        {"x": x[c * B_LOC:(c + 1) * B_LOC], "grids": grids}
        for c in range(N_CORES)
    ]
    res = run_bass_kernel_spmd(
        _PROG, in_maps, core_ids=list(range(N_CORES)), trace=_trace)
    outs = [res.results[c]["out"] for c in range(N_CORES)]
    accs = [res.results[c]["acc"].astype(np.float64) for c in range(N_CORES)]
    output = np.concatenate(outs, axis=0)
    acc = np.stack(accs)                                        # [8,4,128,3]
    S_cnt = acc.sum()

    loss_layer, metrics = _host_assemble(x, targets, S_cnt)
    if _trace:
        kernel._last_exec_ns = res.exec_time_ns
        kernel._last_results = res
    return output, loss_layer, metrics
